# revision 59
# baseline (speedup 1.0000x reference)
"""Causal multi-head attention (B=2, S=2048, D=1024, H=16) on 8 NeuronCores.

Sharding: head-parallel. Core c owns heads {2c, 2c+1} = a 128-wide slice of
the q/k/v projection output dims and of wo's input dim. Each core computes
attention for its 2 heads over both batch elements and a full-size partial
of the final projection; the host sums the 8 partials.

v3 vs v2 (257us -> ~150us/rep steady state, ~182us single-shot):
- causal masking moved onto the TensorE: one tiny extra matmul
  (negI.T @ triu01) accumulates -3e38 onto the masked triangle of the
  diagonal score tile before exp, so exp emits exact zeros and the Pool
  mask-multiply — which head-of-line blocked the exp->av path behind the
  softmax-normalize broadcasts — disappears entirely (this was the single
  biggest win: ~70us).
- softmax reciprocal actually uses reciprocal_approx_fast now (the v2 code
  called the 5x-slower exact op; 3.35us -> 0.66us per row). Its custom uop
  mis-reads any source with base_partition != 0, so the den row is staged
  to SBUF first; the whole normalize chain is emitted grouped and at high
  priority since it gates ps_o reuse.
- one continuous pipeline across benchmark repeats: weights load once;
  qT/kT/vN/oT ring-buffer (bufs=2) so the next rep's projections and x
  prefetch weave into the current rep's late chunks — no PE stall or HAM
  re-throttle at rep boundaries, no per-rep serial prologue.
- x prefetched 3 chunks ahead (one 1MB DMA per chunk, host pre-arranged to
  [sc, p, t, c]); projections run 2 chunks ahead of attention so their
  evacuations land a full chunk before qk needs the diagonal key block.
- fp8 was tried and rejected: under the max-abs-rel gate (2e-2), fp8e4 v
  alone measures 2.7e-2 and fp8 x/w projections 3.0e-2 — the tails of the
  quantization error blow the budget even though rms error is ~1%.

Kernel layout trick: scores are computed *transposed* (scoresT[k, q]), so the
softmax probs come out k-partitioned and feed the attn@v matmul directly.
An extra ones-column appended to v makes the same matmul emit the softmax
denominators. Scores are small (|s/8| < ~3) so softmax without
max-subtraction is exact.
"""
import numpy as np
import ml_dtypes

import concourse.bass as bass
import concourse.tile as tile
from concourse import bacc, mybir
from concourse.bass_utils import run_bass_kernel_spmd
from concourse.masks import make_identity

B, S, D = 2, 2048, 1024
H, HD = 16, 64
NCORES = 8
SF = B * S              # 4096 flattened rows
CH = 512                # column chunk for matmuls
KT = 128                # k-tile (keys per tile)
NEG = -1.0e38

F32 = mybir.dt.float32
F32R = mybir.dt.float32r
BF16 = mybir.dt.bfloat16
BF16NP = ml_dtypes.bfloat16

_cache = {}


ESCALE = 0.125          # 1/sqrt(head_dim)


def _emit_all(nc, tc, io, repeats):
    """One continuous pipeline over repeats*8 globally-indexed q-chunks.

    Weights load once; per-rep scratch (qT/kT/vN/oT) ring-buffers (bufs=2)
    so rep r+1's projections/x-prefetch weave into rep r's late chunks as
    PE fill — no serial prologue or PE stall at repeat boundaries.
    """
    xt, wqt, wkt, wvt, wot, maskt, outp = io
    Exp = mybir.ActivationFunctionType.Exp
    NG = repeats * 8
    order = [(b, qc) for b in range(B) for qc in range(4)]

    with tc.tile_pool(name="shared", bufs=1) as shp, \
         tc.tile_pool(name="scr", bufs=2) as scr, \
         tc.tile_pool(name="pj_ps", bufs=2, space="PSUM") as pj_ps, \
         tc.tile_pool(name="sc_ps", bufs=2, space="PSUM") as sc_ps, \
         tc.tile_pool(name="out_ps", bufs=1, space="PSUM") as out_ps, \
         tc.tile_pool(name="xt_p", bufs=4) as xt_p, \
         tc.tile_pool(name="vt_p", bufs=2) as vt_p, \
         tc.tile_pool(name="exp_p", bufs=6) as exp_p, \
         tc.tile_pool(name="sums_p", bufs=3) as sums_p, \
         tc.tile_pool(name="stg_p", bufs=4) as stg_p:

        wq_s = shp.tile([128, 8, 128], BF16)
        wk_s = shp.tile([128, 8, 128], BF16)
        wv_s = shp.tile([128, 8, 128], BF16)
        wo_s = shp.tile([128, D], BF16)
        # mtile[:, 0:128] = strict upper-tri 0/1 (key > query); [:, 128:256]
        # = -BIG * I. One extra matmul (negI.T @ triu01) accumulates -BIG
        # onto the masked triangle of the diagonal score tile, so exp emits
        # exact zeros and no post-exp mask multiply is needed on any engine.
        mtile = shp.tile([128, 256], BF16)
        ident = shp.tile([128, 128], F32)

        # startup: interleave the first x-chunk's t-blocks with the weight
        # blocks so the first projection matmul has wq_s[0] and xti0[:,0]
        # as early as possible (matters for single-shot latency only)
        xt_r = xt.ap()
        xtiles = {}
        rep_tiles = {}
        xti0 = xt_p.tile([128, 8, CH], BF16, name="xti_0", tag="xti")
        xtiles[0] = xti0
        wq_r = wqt.ap().rearrange("(t p) m -> p t m", p=128)
        nc.sync.dma_start(wq_s[:, 0, :], wq_r[:, 0, :])
        nc.sync.dma_start(xti0[:, 0, :], xt_r[0, :, 0, :])
        nc.sync.dma_start(wq_s[:, 1:8, :], wq_r[:, 1:8, :])
        for t in range(1, 8):
            nc.sync.dma_start(xti0[:, t, :], xt_r[0, :, t, :])
        nc.sync.dma_start(wk_s[:], wkt.ap().rearrange("(t p) m -> p t m", p=128))
        nc.sync.dma_start(wv_s[:], wvt.ap().rearrange("(t p) m -> p t m", p=128))
        nc.sync.dma_start(mtile[:], maskt.ap())
        nc.sync.dma_start(wo_s[:], wot.ap())
        make_identity(nc, ident[:])

        def get_rep(rep):
            if rep not in rep_tiles:
                qT = scr.tile([128, SF], BF16, tag="qT", name=f"qT_{rep}")
                kT = scr.tile([128, SF], BF16, tag="kT", name=f"kT_{rep}")
                # vN half layout (128 wide per head): [ones | 63 zeros |
                # 64 v-dims]. The av matmul then emits the softmax denom at
                # PSUM row 0 — readable by reciprocal_approx_fast directly
                # (its custom uop only works at base_partition 0) — and the
                # attn dims at rows 64..127. Full-128-col stationary also
                # keeps FWL enabled.
                vN = scr.tile([128, 32, 2, 128], BF16, tag="vN",
                              name=f"vN_{rep}")
                oT = scr.tile([128, SF], BF16, tag="oT", name=f"oT_{rep}")
                nc.vector.memset(vN[:, :, :, 0:1], 1.0)
                nc.vector.memset(vN[:, :, :, 1:64], 0.0)
                rep_tiles[rep] = (qT, kT, vN, oT)
            return rep_tiles[rep]

        def load_x(g):
            """Prefetch the x s-chunk for global chunk g as one 1MB DMA
            (host pre-arranged to [sc, p, t, c])."""
            xti = xt_p.tile([128, 8, CH], BF16, name=f"xti_{g}", tag="xti")
            nc.sync.dma_start(xti[:], xt_r[g % 8])
            xtiles[g] = xti

        def proj_steps(g):
            """Projection of global s-chunk g into its rep's qT/kT/vN as a
            list of deferred steps (~2 matmuls each) woven between attention
            k-tiles as PE fill. x must already be prefetched via load_x."""
            sc = g % 8
            qT, kT, vN, oT = get_rep(g // 8)
            box = {}
            steps = []
            col = slice(sc * CH, (sc + 1) * CH)

            def evac(nm, kind):
                ps = box[nm]
                if kind == "q":
                    nc.vector.tensor_copy(qT[:, col], ps[:])
                elif kind == "k":
                    nc.vector.tensor_copy(kT[:, col], ps[:])
                else:
                    vts = vt_p.tile([128, CH], F32,
                                    name=f"vts_{g}", tag="vts")
                    nc.vector.tensor_copy(vts[:], ps[:])
                    # PE transpose per 128-block (dma_start_transpose NaNs
                    # on real HW here):
                    #   vts[hp*64+d, j*128+p] -> vN[p, sc*4+j, hp, 64+d]
                    for j in range(4):
                        tp = pj_ps.tile([128, 128], F32, tag="pj",
                                        padded_shape=[128, CH],
                                        name=f"tp_{g}_{j}")
                        nc.tensor.transpose(
                            tp[:], vts[:, j * 128:(j + 1) * 128], ident[:])
                        nc.vector.tensor_copy(
                            vN[:, sc * 4 + j, :, 64:128],
                            tp[:].rearrange("p (a b) -> p a b", a=2))

            for nm, w_s, kind in (("psq", wq_s, "q"), ("psk", wk_s, "k"),
                                  ("psv", wv_s, "v")):
                for t0 in range(0, 8, 2):
                    def mms(t0=t0, nm=nm, w_s=w_s, kind=kind):
                        if t0 == 0:
                            box[nm] = pj_ps.tile([128, CH], F32, tag="pj",
                                                 name=f"{nm}_{g}")
                        ps = box[nm]
                        for t in (t0, t0 + 1):
                            nc.tensor.matmul(ps[:], w_s[:, t, :],
                                             xtiles[g][:, t, :],
                                             start=(t == 0), stop=(t == 7),
                                             skip_group_check=True)
                        if t0 == 6:
                            evac(nm, kind)
                    steps.append(mms)
            return steps

        def attn_qchunk(g, fill=()):
            """Attention + softmax + normalize for global chunk g. qk runs
            one k-tile ahead of exp/av so the exp latency hides behind the
            next tile's score matmuls; `fill` steps (projection / wo
            matmuls / x prefetch) are woven in between so the PE FIFO never
            stalls at an exp-gated av matmul."""
            b, qc = order[g % 8]
            qT, kT, vN, oT = get_rep(g // 8)
            bcol = b * S
            qsl = slice(bcol + qc * CH, bcol + (qc + 1) * CH)
            nkt = 4 * (qc + 1)
            # row 0 = softmax denominator, rows 64..127 = attn out dims
            ps_o = [out_ps.tile([128, CH], F32, tag=f"ps_o{i}",
                                name=f"ps_o{i}_{g}")
                    for i in range(2)]
            ps_ms = {}

            def qk_tile(kt):
                r0 = max(kt * KT - qc * CH, 0)
                diag = kt * KT - qc * CH >= 0
                ps_m = sc_ps.tile([128, 2, CH], F32, tag="ps_s",
                                  name=f"ps_m_{g}_{kt}")
                ps_ms[kt] = ps_m
                for hp in range(2):
                    hsl = slice(hp * 64, hp * 64 + 64)
                    nc.tensor.matmul(
                        ps_m[:, hp, r0:CH],
                        kT[hsl, bcol + kt * KT: bcol + (kt + 1) * KT],
                        qT[hsl, bcol + qc * CH + r0: bcol + (qc + 1) * CH],
                        start=True, stop=not diag, skip_group_check=True)
                if diag:
                    # accumulate -BIG onto the strict upper triangle of the
                    # diagonal 128 columns: exp then emits exact zeros there
                    for hp in range(2):
                        nc.tensor.matmul(
                            ps_m[:, hp, r0:r0 + 128],
                            mtile[:, 128:256], mtile[:, 0:128],
                            start=False, stop=True, skip_group_check=True)

            def av_tile(kt):
                r0 = max(kt * KT - qc * CH, 0)
                ps_m = ps_ms.pop(kt)
                et = exp_p.tile([128, 2, CH], BF16, tag="et",
                                name=f"et_{g}_{kt}")
                nc.scalar.activation(et[:, :, r0:CH], ps_m[:, :, r0:CH],
                                     Exp, scale=ESCALE)
                for hp in range(2):
                    nc.tensor.matmul(
                        ps_o[hp][:, r0:CH],
                        vN[:, b * 16 + kt, hp, :],
                        et[:, hp, r0:CH],
                        start=(kt == 0), stop=(kt == nkt - 1),
                        skip_group_check=True)

            nfill, fi = len(fill), 0
            qk_tile(0)
            for kt in range(nkt):
                if kt + 1 < nkt:
                    qk_tile(kt + 1)
                # spread fill steps evenly over the k-tiles, between the
                # lookahead qk and the exp-gated av
                want = ((kt + 1) * nfill) // nkt
                while fi < want:
                    fill[fi]()
                    fi += 1
                av_tile(kt)
            # softmax normalize: den sits at PSUM row 0 (base_partition 0 —
            # the only place reciprocal_approx_fast's custom uop reads
            # correctly), so the reciprocal reads PSUM directly. All 2-hp
            # DVE ops are emitted grouped so the Pool broadcasts don't
            # head-of-line block the DVE FIFO; the chain runs at high
            # priority because it gates ps_o reuse by the next chunk's
            # first av matmul.
            with tc.high_priority():
                rrows, bcs = [], []
                for hp in range(2):
                    rrow = sums_p.tile([1, CH], F32, tag="rrow",
                                       name=f"rrow_{g}_{hp}")
                    nc.vector.reciprocal_approx_fast(rrow[:],
                                                     ps_o[hp][0:1, :])
                    rrows.append(rrow)
                for hp in range(2):
                    bc = sums_p.tile([64, CH], F32, tag="bc",
                                     name=f"bc_{g}_{hp}")
                    nc.gpsimd.partition_broadcast(bc[:], rrows[hp][0:1, :])
                    bcs.append(bc)
                for hp in range(2):
                    nc.vector.tensor_mul(
                        oT[hp * 64: hp * 64 + 64, qsl],
                        ps_o[hp][64:128, :], bcs[hp][:])

        def wo_steps(g, use_act=False):
            """Final projection partial for global chunk g as deferred steps
            (one matmul + evacuation each); run one chunk behind attention
            so the oT normalize chain has time to finish. With use_act,
            evacuations alternate DVE/ACT (for the bare tail where ACT has
            no exp work and DVE paces the PSUM ring)."""
            b, qc = order[g % 8]
            _, _, _, oT = get_rep(g // 8)
            bcol = b * S
            box = {}
            steps = []
            for st4 in range(4):
                for chn in range(2):
                    def step(st4=st4, chn=chn):
                        soff = bcol + qc * CH + st4 * 128
                        if chn == 0:
                            box[st4] = stg_p.tile(
                                [128, D], BF16, tag="stg",
                                name=f"stg_{g}_{st4}")
                        stg = box[st4]
                        psf = pj_ps.tile(
                            [128, CH], F32, tag="pj",
                            name=f"psf_{g}_{st4}_{chn}")
                        nc.tensor.matmul(psf[:],
                                         oT[:, soff: soff + 128],
                                         wo_s[:, chn * CH:(chn + 1) * CH],
                                         start=True, stop=True,
                                         skip_group_check=True)
                        dst = stg[:, chn * CH:(chn + 1) * CH]
                        if use_act and (st4 * 2 + chn) % 2 == 1:
                            nc.scalar.copy(dst, psf[:])
                        else:
                            nc.vector.tensor_copy(dst, psf[:])
                        if chn == 1:
                            nc.sync.dma_start(outp.ap()[soff: soff + 128, :],
                                              stg[:])
                    steps.append(step)
            return steps

        # pipeline: x prefetch 3 chunks ahead, proj 2 chunks ahead, wo one
        # chunk behind — all woven between attention k-tiles as PE fill and
        # crossing repeat boundaries without a stall. (x chunk 0 already
        # loading, interleaved with the weights above.)
        load_x(1)
        load_x(2)
        for s in proj_steps(0) + proj_steps(1):
            s()
        for g in range(NG):
            fill = []
            if g + 3 < NG:
                fill.append(lambda gg=g + 3: load_x(gg))
            if g + 2 < NG:
                fill += proj_steps(g + 2)
            if g >= 1:
                fill += wo_steps(g - 1)
            attn_qchunk(g, fill)
        for s in wo_steps(NG - 1, use_act=True):
            s()


def _build(repeats=1):
    nc = bacc.Bacc("TRN2", target_bir_lowering=False, debug=False)
    xt = nc.dram_tensor("xt", [SF // CH, 128, 8, CH], BF16, kind="ExternalInput")
    wqt = nc.dram_tensor("wqt", [D, 128], BF16, kind="ExternalInput")
    wkt = nc.dram_tensor("wkt", [D, 128], BF16, kind="ExternalInput")
    wvt = nc.dram_tensor("wvt", [D, 128], BF16, kind="ExternalInput")
    wot = nc.dram_tensor("wot", [128, D], BF16, kind="ExternalInput")
    maskt = nc.dram_tensor("maskt", [128, 256], BF16, kind="ExternalInput")
    outp = nc.dram_tensor("outp", [SF, D], BF16, kind="ExternalOutput")
    io = (xt, wqt, wkt, wvt, wot, maskt, outp)

    with tile.TileContext(nc) as tc:
        _emit_all(nc, tc, io, repeats)
    nc.compile()
    return nc


def _causal_mask_tile() -> np.ndarray:
    # [128, 0:128]: strict upper-tri 0/1 (1 where key kp > query c);
    # [128, 128:256]: -BIG * identity. negI.T @ triu01 accumulated onto the
    # diagonal score tile drives masked scores to -BIG before exp.
    kp = np.arange(128)[:, None]
    c = np.arange(128)[None, :]
    triu01 = (kp > c).astype(BF16NP)
    negI = (np.eye(128) * np.float32(-3.0e38)).astype(BF16NP)
    return np.concatenate([triu01, negI], axis=1)


def make_in_maps(x, wq, wk, wv, wo):
    # xt_arr[sc, p, t, s] = x[sc*CH + s, t*128 + p] — each sc block is a
    # contiguous 1MB single-DMA source whose element order matches the SBUF
    # dest tile [p, t, s]
    xt = np.ascontiguousarray(
        x.reshape(SF // CH, CH, 8, 128).transpose(0, 3, 2, 1).astype(BF16NP))
    mask = _causal_mask_tile()
    in_maps = []
    for c in range(NCORES):
        rows = slice(c * 128, (c + 1) * 128)
        in_maps.append({
            "xt": xt,
            "wqt": np.ascontiguousarray(wq[rows, :].T.astype(BF16NP)),
            "wkt": np.ascontiguousarray(wk[rows, :].T.astype(BF16NP)),
            "wvt": np.ascontiguousarray(wv[rows, :].T.astype(BF16NP)),
            "wot": np.ascontiguousarray(wo[:, rows].T.astype(BF16NP)),
            "maskt": mask,
        })
    return in_maps


def _make_runner(nc):
    """Build a cached jitted PJRT runner. xt/maskt are replicated (same data
    on every core); weight slices are sharded per core; outputs unsharded on
    host. No donation: the zero output-init buffers stay resident on device
    across calls (the kernel writes every output element)."""
    import jax
    from jax.sharding import Mesh, PartitionSpec, NamedSharding
    try:
        from jax.experimental.shard_map import shard_map
    except ImportError:
        shard_map = jax.shard_map
    from concourse.bass2jax import (_bass_exec_p, install_neuronx_cc_hook,
                                    partition_id_tensor)

    install_neuronx_cc_hook()
    pname = nc.partition_id_tensor.name if nc.partition_id_tensor else None
    in_names, out_names, out_avals, zero_shapes = [], [], [], []
    for alloc in nc.m.functions[0].allocations:
        if not isinstance(alloc, mybir.MemoryLocationSet):
            continue
        name = alloc.memorylocations[0].name
        if alloc.kind == "ExternalInput":
            if name != pname:
                in_names.append(name)
        elif alloc.kind == "ExternalOutput":
            out_names.append(name)
            shape = tuple(alloc.tensor_shape)
            dtype = mybir.dt.np(alloc.dtype)
            out_avals.append(jax.core.ShapedArray(shape, dtype))
            zero_shapes.append((shape, dtype))
    all_in_names = in_names + out_names
    if pname is not None:
        all_in_names = all_in_names + [pname]

    def _body(*args):
        operands = list(args)
        if pname is not None:
            operands.append(partition_id_tensor())
        return tuple(_bass_exec_p.bind(
            *operands,
            out_avals=tuple(out_avals),
            in_names=tuple(all_in_names),
            out_names=tuple(out_names),
            lowering_input_output_aliases=(),
            sim_require_finite=True,
            sim_require_nnan=True,
            nc=nc,
        ))

    devices = jax.devices()[:NCORES]
    mesh = Mesh(np.asarray(devices), ("core",))
    shard = PartitionSpec("core")
    repl = PartitionSpec()
    REPLICATED = ("xt", "maskt")
    in_specs = tuple(repl if n in REPLICATED else shard for n in in_names) \
        + (shard,) * len(out_names)
    sharded = jax.jit(
        shard_map(_body, mesh=mesh, in_specs=in_specs,
                  out_specs=(shard,) * len(out_names), check_rep=False),
        keep_unused=True)
    zeros = [jax.device_put(np.zeros((NCORES * s[0], *s[1:]), d),
                            NamedSharding(mesh, shard))
             for (s, d) in zero_shapes]
    jax.block_until_ready(zeros)

    def run(in_maps):
        args = []
        for n in in_names:
            if n in REPLICATED:
                args.append(jax.device_put(np.asarray(in_maps[0][n]),
                                           NamedSharding(mesh, repl)))
            else:
                args.append(jax.device_put(
                    np.concatenate([np.asarray(m[n]) for m in in_maps], axis=0),
                    NamedSharding(mesh, shard)))
        outs = sharded(*args, *zeros)
        return [
            {n: np.asarray(outs[i]).reshape(NCORES, *out_avals[i].shape)[c]
             for i, n in enumerate(out_names)}
            for c in range(NCORES)
        ]

    return run


def kernel(x, wq, wk, wv, wo):
    x = np.asarray(x, dtype=np.float32)
    wq = np.asarray(wq, dtype=np.float32)
    wk = np.asarray(wk, dtype=np.float32)
    wv = np.asarray(wv, dtype=np.float32)
    wo = np.asarray(wo, dtype=np.float32)

    if "nc" not in _cache:
        _cache["nc"] = _build()
    nc = _cache["nc"]
    in_maps = make_in_maps(x, wq, wk, wv, wo)

    try:
        if "run" not in _cache:
            _cache["run"] = _make_runner(nc)
        results = _cache["run"](in_maps)
    except Exception:
        _cache.pop("run", None)
        results = run_bass_kernel_spmd(
            nc, in_maps, core_ids=list(range(NCORES))).results

    out = np.zeros((SF, D), dtype=np.float64)
    for r in results:
        out += r["outp"].astype(np.float64)
    return out.astype(np.float32).reshape(B, S, D)

